# revision 3
# baseline (speedup 1.0000x reference)
"""Trainium2 Bass kernel v2 for CFGSubASTExpressionCombiner.

Segment-softmax attention over sub-ASTs grouped by PDG node. Contract:
kernel(**inputs) takes FULL unsharded numpy inputs, returns the FULL
[N_PDG, D] output.

Sharding / host prep (the "graph partitioning" of the sharding hint):
  - Elements (E-length arrays) are sorted by segment; segments are split
    into 8 contiguous ranges of 6250, one per core; each core's segments
    are processed in 49 blocks of 128.
  - The element AST rows are PRE-GATHERED on the host into a per-core
    bf16 stream laid out p-major ([128, n_cols*257]: col c = element tile
    c = b*t_b + t, last lane of each 257 group is the ones column used to
    accumulate the softmax denominator). The device then does only bulk
    sequential DMA — no indirect gathers, no descriptor-generation
    bottleneck on the Q7.
  - Query rows (sub-AST roots) are pre-gathered AND pre-transposed into
    bf16 qT tiles ([128, n_blocks*256], chunk k of block b holds
    root_enc[segs of b, 128k:128(k+1)].T).
  - Wk^T*scale and Wv ship replicated in bf16 as two 128-row K-chunks.

Device math per block b (128 segments, t_b element tiles):
  qkT[d1,seg] = sum_d2 wkt_s[d2,d1] qT[d2,seg]       (4 bf16 matmuls)
  per element tile t: xT = PE-transpose(x)           (2 bf16 transposes)
    S[e,seg]  = xT.T @ qkT                           (2 bf16 matmuls, PSUM)
    pmat      = exp(S) * [slid(e)==seg]              (scalar EXP + DVE mask)
    U[seg,:]  += pmat.T @ [x|1]                      (1 bf16 matmul, PSUM acc)
  out[seg,:] = (U[:, :256]/max(U[:,256],eps)) @ Wv   (scalar scale + 2 matmuls)

All matmuls bf16 (1 cycle/row on PE vs 4 for fp32); accumulation stays in
fp32 PSUM; softmax numerator/denominator use the SAME bf16-rounded pmat so
the normalization is exact. No max-subtraction: scores ~N(0,1), exp safe.
"""

import math

import numpy as np
import ml_dtypes

import concourse.bass as bass
import concourse.bacc as bacc
import concourse.mybir as mybir
import concourse.tile as tile
from concourse.bass_utils import run_bass_kernel_spmd
from concourse.masks import make_identity

P = 128
D = 256
TC = D + 1  # element tile cols: 256 features + ones column
N_CORES = 8

N_AST_FULL = 500000
N_PDG_FULL = 50000
SEGS_PER_CORE_FULL = N_PDG_FULL // N_CORES          # 6250
N_BLOCKS_FULL = math.ceil(SEGS_PER_CORE_FULL / P)   # 49
T_B_MIN = 8

f32 = mybir.dt.float32
bf16 = mybir.dt.bfloat16
i32 = mybir.dt.int32
BF = ml_dtypes.bfloat16


def _tile_widths(t_b, ship_every):
    """Per-tile stream widths: 257 ([x|1]) or 513 ([x|1|xT])."""
    return [
        (TC + D) if (ship_every and t % ship_every == 0) else TC
        for t in range(t_b)
    ]


def _build_nc(n_blocks, t_b, reps=1, xp_bufs=3, blk_bufs=2, ind_eng="vector",
              ship_every=0, sm_bufs=3, pt_bufs=2, mm_bufs=None, s_bufs=None):
    mm_bufs = pt_bufs if mm_bufs is None else mm_bufs
    s_bufs = pt_bufs if s_bufs is None else s_bufs
    n_cols = n_blocks * t_b
    widths = _tile_widths(t_b, ship_every)
    offs = [0]
    for w in widths[:-1]:
        offs.append(offs[-1] + w)
    band = sum(widths)
    EXP = mybir.ActivationFunctionType.Exp
    COPY = mybir.ActivationFunctionType.Copy

    nc = bacc.Bacc()
    xs = nc.declare_dram_parameter("xs", [P, n_blocks * band], bf16, isOutput=False)
    slid = nc.declare_dram_parameter("slid", [P, n_cols], f32, isOutput=False)
    qt = nc.declare_dram_parameter("qt", [P, n_blocks * D], bf16, isOutput=False)
    wk2 = nc.declare_dram_parameter("wk2", [P, 2 * D], bf16, isOutput=False)
    wv2 = nc.declare_dram_parameter("wv2", [P, 2 * D], bf16, isOutput=False)
    out = nc.declare_dram_parameter("out", [n_blocks * P, D], f32, isOutput=True)

    with tile.TileContext(nc) as tc:
        with (
            tc.tile_pool(name="const", bufs=1) as cpool,
            tc.tile_pool(name="blk", bufs=blk_bufs) as bpool,
            tc.tile_pool(name="xp", bufs=xp_bufs) as xpool,
            tc.tile_pool(name="sm", bufs=sm_bufs) as spool,
            tc.tile_pool(name="pmm", bufs=mm_bufs, space="PSUM") as pmm,
            tc.tile_pool(name="ptr", bufs=pt_bufs, space="PSUM") as ptr,
            tc.tile_pool(name="psp", bufs=s_bufs, space="PSUM") as psp,
            tc.tile_pool(name="pu", bufs=2, space="PSUM") as pu,
        ):
            wk_sb = cpool.tile([P, 2 * D], bf16)
            nc.sync.dma_start(out=wk_sb[:], in_=wk2[:])
            wv_sb = cpool.tile([P, 2 * D], bf16)
            nc.sync.dma_start(out=wv_sb[:], in_=wv2[:])
            ident = cpool.tile([P, P], bf16)
            make_identity(nc, ident[:])
            iota_i = cpool.tile([P, P], i32)
            nc.gpsimd.iota(iota_i[:], pattern=[[1, P]], base=0, channel_multiplier=0)
            iota_f = cpool.tile([P, P], f32)
            nc.vector.tensor_copy(iota_f[:], iota_i[:])
            sl_all = cpool.tile([P, n_cols], f32)
            nc.sync.dma_start(out=sl_all[:], in_=slid[:])

            for _rep in range(reps):
              for b in range(n_blocks):
                # ---- query side: qkT[d1, seg] for this 128-seg block ----
                qtt = bpool.tile([P, D], bf16, tag="qt")
                nc.sync.dma_start(out=qtt[:], in_=qt[:, b * D : (b + 1) * D])
                qk_ps = pmm.tile([P, D], f32, tag="mm")
                for m in range(2):
                    for k in range(2):
                        nc.tensor.matmul(
                            qk_ps[:, m * P : (m + 1) * P],
                            lhsT=wk_sb[:, k * D + m * P : k * D + (m + 1) * P],
                            rhs=qtt[:, k * P : (k + 1) * P],
                            start=(k == 0),
                            stop=(k == 1),
                        )
                qkT = bpool.tile([P, D], bf16, tag="qk")
                nc.vector.tensor_copy(qkT[:], qk_ps[:])

                # ---- element stream: one bulk DMA per block ----
                xblk = xpool.tile([P, band], bf16, tag="xb")
                nc.sync.dma_start(
                    out=xblk[:], in_=xs[:, b * band : (b + 1) * band]
                )

                u_ps = pu.tile([P, TC], f32, tag="u")
                for t in range(t_b):
                    c = b * t_b + t
                    o = offs[t]
                    x = xblk[:, o : o + TC]
                    if widths[t] == TC + D:
                        xT = xblk[:, o + TC : o + TC + D]
                    else:
                        xT_ps = ptr.tile([P, D], bf16, tag="tr")
                        nc.tensor.transpose(xT_ps[:, 0:P], x[:, 0:P], ident[:])
                        nc.tensor.transpose(xT_ps[:, P:D], x[:, P:D], ident[:])
                        xT_sb = spool.tile([P, D], bf16, tag="xT")
                        nc.vector.tensor_copy(xT_sb[:], xT_ps[:])
                        xT = xT_sb[:]

                    s_ps = psp.tile([P, P], f32, tag="s")
                    for k in range(2):
                        nc.tensor.matmul(
                            s_ps[:],
                            lhsT=xT[:, k * P : (k + 1) * P],
                            rhs=qkT[:, k * P : (k + 1) * P],
                            start=(k == 0),
                            stop=(k == 1),
                        )
                    ptil = spool.tile([P, P], bf16, tag="pt")
                    nc.scalar.activation(ptil[:], s_ps[:], EXP)
                    ind = spool.tile([P, P], bf16, tag="ind")
                    ind_e = nc.vector if ind_eng == "vector" else nc.gpsimd
                    ind_e.tensor_scalar(
                        out=ind[:],
                        in0=iota_f[:],
                        scalar1=sl_all[:, c : c + 1],
                        scalar2=None,
                        op0=mybir.AluOpType.is_equal,
                    )
                    pmat = spool.tile([P, P], bf16, tag="pm")
                    nc.vector.tensor_mul(pmat[:], ptil[:], ind[:])
                    nc.tensor.matmul(
                        u_ps[:],
                        lhsT=pmat[:],
                        rhs=x[:],
                        start=(t == 0),
                        stop=(t == t_b - 1),
                    )

                # ---- finalize: out = (U/Z) @ Wv ----
                z = bpool.tile([P, 1], f32, tag="z")
                nc.vector.tensor_scalar_max(z[:], u_ps[:, D : D + 1], 1e-30)
                rz = bpool.tile([P, 1], f32, tag="rz")
                nc.vector.reciprocal(rz[:], z[:])
                up = bpool.tile([P, D], bf16, tag="up")
                nc.scalar.activation(up[:], u_ps[:, 0:D], COPY, scale=rz[:, :1])
                upT_ps = ptr.tile([P, D], bf16, tag="tr")
                nc.tensor.transpose(upT_ps[:, 0:P], up[:, 0:P], ident[:])
                nc.tensor.transpose(upT_ps[:, P:D], up[:, P:D], ident[:])
                upT = bpool.tile([P, D], bf16, tag="upT")
                nc.vector.tensor_copy(upT[:], upT_ps[:])
                f_ps = pmm.tile([P, D], f32, tag="mm")
                for k in range(2):
                    nc.tensor.matmul(
                        f_ps[:],
                        lhsT=upT[:, k * P : (k + 1) * P],
                        rhs=wv_sb[:, k * D : (k + 1) * D],
                        start=(k == 0),
                        stop=(k == 1),
                    )
                o = bpool.tile([P, D], f32, tag="o")
                nc.vector.tensor_copy(o[:], f_ps[:])
                nc.sync.dma_start(out=out[b * P : (b + 1) * P, :], in_=o[:])
    nc.finalize()
    return nc


DEFAULT_SHIP_EVERY = 2
DEFAULT_CFG = dict(ship_every=DEFAULT_SHIP_EVERY)

_NC_CACHE = {}


def _get_nc(n_ast, n_blocks, t_b, reps=1, xp_bufs=3, blk_bufs=2, ind_eng="vector",
            ship_every=0, sm_bufs=3, pt_bufs=2, mm_bufs=None, s_bufs=None):
    key = (n_blocks, t_b, reps, xp_bufs, blk_bufs, ind_eng, ship_every,
           sm_bufs, pt_bufs, mm_bufs, s_bufs)
    if key not in _NC_CACHE:
        _NC_CACHE[key] = _build_nc(
            n_blocks, t_b, reps=reps, xp_bufs=xp_bufs, blk_bufs=blk_bufs,
            ind_eng=ind_eng, ship_every=ship_every, sm_bufs=sm_bufs,
            pt_bufs=pt_bufs, mm_bufs=mm_bufs, s_bufs=s_bufs,
        )
    return _NC_CACHE[key]


def prepare_in_maps(
    ast_nodes_encodings, Wk, Wv, ast_to_pdg_key, ast_to_pdg_value,
    pdg_to_root_key, pdg_to_root_value, nr_cfg_nodes, ship_every=None,
):
    if ship_every is None:
        ship_every = DEFAULT_SHIP_EVERY
    """Host prep: sort by segment, partition, pre-gather + bf16-cast element
    and (transposed) query rows, lay out p-major per core."""
    ast_np = np.ascontiguousarray(np.asarray(ast_nodes_encodings, dtype=np.float32))
    wk_np = np.asarray(Wk, dtype=np.float32)
    wv_np = np.asarray(Wv, dtype=np.float32)
    scale = np.float32(1.0 / np.sqrt(ast_np.shape[1]))
    wkt_s = np.ascontiguousarray(wk_np.T * scale)  # [d2, d1]

    n_pdg = int(nr_cfg_nodes)
    assert ast_np.shape == (N_AST_FULL, D) and n_pdg == N_PDG_FULL
    segs_per_core, n_blocks = SEGS_PER_CORE_FULL, N_BLOCKS_FULL
    seg_slots = n_blocks * P

    ast_to_pdg_key = np.asarray(ast_to_pdg_key)
    ast_to_pdg_value = np.asarray(ast_to_pdg_value)
    order = np.argsort(ast_to_pdg_value, kind="stable")
    seg_sorted = ast_to_pdg_value[order]
    gid_sorted = ast_to_pdg_key[order].astype(np.int64)
    counts = np.bincount(seg_sorted, minlength=n_pdg)
    cum = np.concatenate([[0], np.cumsum(counts)]).astype(np.int64)

    root_full = np.zeros(n_pdg, dtype=np.int64)
    root_full[np.asarray(pdg_to_root_key)] = np.asarray(pdg_to_root_value)

    # global t_b: elements of any 128-seg block must fit t_b*128 slots
    block_max = 0
    for c in range(N_CORES):
        s0 = c * segs_per_core
        s1 = min(s0 + segs_per_core, n_pdg)
        bs = np.arange(s0, s1, P)
        be = np.minimum(bs + P, s1)
        block_max = max(block_max, int((cum[be] - cum[bs]).max()))
    t_b = max(T_B_MIN, math.ceil(block_max / P))
    n_cols = n_blocks * t_b

    # weights, replicated
    wk2 = np.zeros((P, 2 * D), dtype=BF)
    wk2[:, 0:D] = wkt_s[0:P, :].astype(BF)
    wk2[:, D : 2 * D] = wkt_s[P : 2 * P, :].astype(BF)
    wv2 = np.zeros((P, 2 * D), dtype=BF)
    wv2[:, 0:D] = wv_np[0:P, :].astype(BF)
    wv2[:, D : 2 * D] = wv_np[P : 2 * P, :].astype(BF)

    in_maps = []
    for c in range(N_CORES):
        s0 = c * segs_per_core
        s1 = min(s0 + segs_per_core, n_pdg)

        # per-slot gather index + local segment id (slot = c_col*128 + p)
        gidx = np.zeros(n_cols * P, dtype=np.int64)
        slid = np.full(n_cols * P, -1.0, dtype=np.float32)
        for b in range(n_blocks):
            bs0 = s0 + b * P
            bs1 = min(bs0 + P, s1)
            if bs0 >= bs1:
                continue
            be0, be1 = cum[bs0], cum[bs1]
            n_b = be1 - be0
            if n_b > t_b * P:
                raise OverflowError(n_b)
            o0 = b * t_b * P
            gidx[o0 : o0 + n_b] = gid_sorted[be0:be1]
            slid[o0 : o0 + n_b] = (seg_sorted[be0:be1] - bs0).astype(np.float32)

        # pre-gather element rows -> bf16, p-major banded stream
        # (per tile: [x|1] and optionally the pre-transposed xT chunks)
        widths = _tile_widths(t_b, ship_every)
        offs = np.concatenate([[0], np.cumsum(widths)[:-1]]).astype(int)
        band = int(sum(widths))
        xg_all = ast_np[gidx].astype(BF).reshape(n_cols, P, D)  # [c, p(e), d]
        xs_pm = np.ones((P, n_blocks * band), dtype=BF)
        for b in range(n_blocks):
            for t in range(t_b):
                cc = b * t_b + t
                o = b * band + int(offs[t])
                xt = xg_all[cc]
                xs_pm[:, o : o + D] = xt
                if widths[t] == TC + D:
                    xs_pm[:, o + TC : o + TC + D] = (
                        xt.reshape(P, 2, P).transpose(2, 1, 0).reshape(P, D)
                    )

        slid_pm = np.ascontiguousarray(slid.reshape(n_cols, P).T)

        # query side: root rows for this core's segs, transposed per block
        roots = np.zeros(seg_slots, dtype=np.int64)
        roots[: s1 - s0] = root_full[s0:s1]
        qrows = ast_np[roots].astype(BF)            # [seg_slots, 256]
        qrows[s1 - s0 :, :] = 0
        # qt_pm[p, b*256 + k*128 + j] = qrows[b*128 + j, k*128 + p]
        qt_pm = np.ascontiguousarray(
            qrows.reshape(n_blocks, P, 2, P).transpose(3, 0, 2, 1)
        ).reshape(P, n_blocks * D)  # [p, b*256 + k*128 + j]

        in_maps.append({
            "xs": xs_pm,
            "slid": slid_pm,
            "qt": np.ascontiguousarray(qt_pm),
            "wk2": wk2,
            "wv2": wv2,
        })
    return in_maps, n_blocks, t_b


def kernel(
    ast_nodes_encodings, Wk, Wv, ast_to_pdg_key, ast_to_pdg_value,
    pdg_to_root_key, pdg_to_root_value, nr_cfg_nodes,
):
    in_maps, n_blocks, t_b = prepare_in_maps(
        ast_nodes_encodings, Wk, Wv, ast_to_pdg_key, ast_to_pdg_value,
        pdg_to_root_key, pdg_to_root_value, nr_cfg_nodes,
    )
    n_pdg = int(nr_cfg_nodes)
    nc = _get_nc(N_AST_FULL, n_blocks, t_b, **DEFAULT_CFG)
    res = run_bass_kernel_spmd(nc, in_maps, list(range(N_CORES)))

    full = np.zeros((n_pdg, D), dtype=np.float32)
    segs_per_core = SEGS_PER_CORE_FULL
    for c in range(N_CORES):
        s0 = c * segs_per_core
        s1 = min(s0 + segs_per_core, n_pdg)
        full[s0:s1] = res.results[c]["out"][: s1 - s0]
    return full
